# revision 33
# baseline (speedup 1.0000x reference)
"""Multi-head self-attention Trainium2 kernel.

Problem: x[2, 2048, 768] -> MHSA (12 heads, head_dim 64) -> out[2, 2048, 768].

Sharding over 8 NeuronCores: core c handles batch c//4 and heads
[3*(c%4), 3*(c%4)+3). Each core computes its 3 heads' attention and a
row-split partial of the output projection over its 192 channels (bf16
partials); the host sums the 4 partials per batch and transposes.

The ScalarE exp stream (96 activations, ~110us) is the bottleneck; the
kernel is a just-in-time pipeline that keeps it saturated:

- per chunk c: phase A emits h0/h1 score groups (4 quadrant-paired matmuls
  -> 2 exps into one [128,2048] pt tile) and runs attn@V(h0,h1) two groups
  behind (lag-2 so attn@V never blocks the in-order TensorE queue on an
  exp it consumes); phase B emits h2 score groups, runs attn@V(h2) lag-2
  and the previous chunk's projection.
- chunk 0's phase A slots carry the QKV projections (512-col granularity
  on the idle proj PSUM banks) and V-natural tiles; DMA issue order is
  arranged so no early matmul head-of-line-blocks the queue on a late DMA.
- chunk 3 runs h0/h1 first so only the short h2 divide chain and the
  final projection (pre-issued h01 matmuls, pairs on idle score banks)
  trail the last exp.
- softmax row-sums ride attn@V as an appended ones-column; exp computes
  exp(s/8 - 2) (shift-invariant); the divide is a 128-lane reciprocal
  (reshape DMAs) + GPSIMD partition broadcast (dest must be partition 0;
  upper half via DMA hop).
- projection packs heads 0,1 into one K=128 matmul (ao stacked [128,seq])
  and quadrant-pairs the two K=64 h2 matmuls across banks.
- PSUM: 4 banks score ring (2x[128,1024]) + 4 single-bank accumulators
  (pv0, pv1, pv2/proj-even, proj-odd).
- a dummy-matmul burst at t=0 warms the PE HAM clock gate; a warmup
  activation preloads the exp table during the input DMA phase.
"""

import sys

sys.path.insert(0, "/opt/trn_rl_repo")

import numpy as np

EMBED = 768
N_SEQ = 2048
HD = 64
N_CORES = 8
KT = EMBED // 128  # 6 contraction tiles
MT = N_SEQ // 128  # 16 key tiles
QCH = 512
NCH = N_SEQ // QCH  # 4 chunks
NG = MT // 2  # 8 groups of 2 key tiles
VW = 208  # per-j V block width in vtp (3*65 ones-interleaved + pad)

_CACHED = {}


def _build(debug=False, no_proj=False, no_dup=False, no_spill=False, no_c3r=False):
    from concourse import bacc
    import concourse.tile as tile
    import concourse.mybir as mybir

    F32 = mybir.dt.float32
    BF16 = mybir.dt.bfloat16
    EXP = mybir.ActivationFunctionType.Exp

    nc = bacc.Bacc()
    xT = nc.declare_dram_parameter("xT", [EMBED, N_SEQ], BF16, isOutput=False)
    # Q/K weights in 3 m-tiles of 128 cols: [Qh0|Qh1], [Kh0|Kh1], [Qh2|Kh2]
    wqk = nc.declare_dram_parameter("wqk", [EMBED, 384], BF16, isOutput=False)
    wv = nc.declare_dram_parameter("wv", [EMBED, 192], BF16, isOutput=False)
    wp01 = nc.declare_dram_parameter("wp01", [128, EMBED], BF16, isOutput=False)
    wp2 = nc.declare_dram_parameter("wp2", [128, EMBED], BF16, isOutput=False)
    outT = nc.declare_dram_parameter("outT", [EMBED, N_SEQ], BF16, isOutput=True)
    if debug:
        dbg_qk = nc.declare_dram_parameter("dbg_qk", [4, 128, N_SEQ], BF16, isOutput=True)
        dbg_vt = nc.declare_dram_parameter("dbg_vt", [NG, 128, 2 * VW], BF16, isOutput=True)
        dbg_ao = nc.declare_dram_parameter("dbg_ao", [2, 128, N_SEQ], BF16, isOutput=True)

    with tile.TileContext(nc) as tc:
        with (
            tc.tile_pool(name="persist", bufs=1) as pp,
            tc.tile_pool(name="pt", bufs=10) as ptp,
            tc.tile_pool(name="work", bufs=2) as wk,
            tc.tile_pool(name="otp", bufs=6) as otp,
            tc.tile_pool(name="psS", bufs=2, space="PSUM") as psS,
            tc.tile_pool(name="psA", bufs=1, space="PSUM") as psA,
            tc.tile_pool(name="psB", bufs=1, space="PSUM") as psB,
            tc.tile_pool(name="psC", bufs=1, space="PSUM") as psC,
            tc.tile_pool(name="psD", bufs=1, space="PSUM") as psD,
        ):
            qk = [
                pp.tile([128, N_SEQ], BF16, tag=f"qk{m}", name=f"qk{m}")
                for m in range(3)
            ]
            qk2d = pp.tile([128, N_SEQ], BF16, tag="qk2d")
            vtp = [
                pp.tile([128, 2 * VW], BF16, tag=f"vt{g}", name=f"vt{g}")
                for g in range(NG)
            ]
            xt = [
                pp.tile([128, N_SEQ], BF16, tag=f"xt{k}", name=f"xt{k}")
                for k in range(KT)
            ]
            wqk_t = [
                pp.tile([128, 384], BF16, tag=f"wqk{k}", name=f"wqkt{k}")
                for k in range(KT)
            ]
            wv_t = pp.tile([128, KT * 192], BF16, tag="wv")
            wp01_t = pp.tile([128, EMBED], BF16, tag="wp01")
            wp2_t = pp.tile([128, EMBED], BF16, tag="wp2")
            ao01 = pp.tile([128, N_SEQ], BF16, tag="ao01")
            ao2 = pp.tile([128, N_SEQ], BF16, tag="ao2")
            warm0 = pp.tile([128, 8], F32, tag="warm0")
            wz = pp.tile([128, QCH], BF16, tag="wz")
            warm1 = pp.tile([128, 8], BF16, tag="warm1")
            nbias = pp.tile([128, 1], F32, tag="nbias")

            xT_ap = xT[:, :].rearrange("(t p) n -> t p n", p=128)
            wqk_ap = wqk[:, :].rearrange("(t p) n -> t p n", p=128)
            wv_ap = wv[:, :].rearrange("(t p) n -> t p n", p=128)

            # ---- DMA issue order == Sync queue order ----
            for k in range(KT):
                nc.sync.dma_start(out=wqk_t[k], in_=wqk_ap[k])
                nc.sync.dma_start(out=xt[k][:, 0:QCH], in_=xT_ap[k][:, 0:QCH])
            for k in range(KT):
                nc.sync.dma_start(
                    out=xt[k][:, QCH : 2 * QCH], in_=xT_ap[k][:, QCH : 2 * QCH]
                )
            for k in range(KT):
                nc.sync.dma_start(
                    out=wv_t[:, k * 192 : (k + 1) * 192], in_=wv_ap[k]
                )
            for k in range(KT):
                nc.sync.dma_start(
                    out=xt[k][:, 2 * QCH :], in_=xT_ap[k][:, 2 * QCH :]
                )
            nc.sync.dma_start(out=wp01_t, in_=wp01[:, :])
            nc.sync.dma_start(out=wp2_t, in_=wp2[:, :])

            # ones columns of vtp via DVE (keep the Sync queue clear)
            for g in range(NG):
                for j in range(2):
                    oap = (
                        vtp[g][:, VW * j : VW * j + 195]
                        .rearrange("p (h c) -> p h c", c=65)[:, :, 64]
                    )
                    nc.vector.memset(oap, 1.0)
            # warmup: pull the exp table load off the critical path
            nc.vector.memset(warm0, 0.0)
            nc.vector.memset(nbias, -2.0)
            nc.scalar.activation(out=warm1, in_=warm0, func=EXP, scale=0.125)

            # ---- building blocks ----
            cdflip = [0]

            def cd_tile():
                pool, tag = (psC, "C") if cdflip[0] == 0 else (psD, "D")
                cdflip[0] ^= 1
                return pool.tile([128, QCH], F32, tag=tag, name="pcd")

            def qk_mtileH(m, h):
                """QKV projection m-tile over 512 cols [h*512, +512)."""
                ps = cd_tile()
                lo = h * QCH
                for k in range(KT):
                    nc.tensor.matmul(
                        ps,
                        wqk_t[k][:, m * 128 : (m + 1) * 128],
                        xt[k][:, lo : lo + QCH],
                        start=(k == 0),
                        stop=(k == KT - 1),
                    )
                nc.vector.tensor_copy(out=qk[m][:, lo : lo + QCH], in_=ps)

            def scores01(c, g):
                """h0/h1 score tiles for key tiles 2g,2g+1; exp into one pt."""
                qs = slice(c * QCH, (c + 1) * QCH)
                s0 = psS.tile([128, 2 * QCH], F32, tag="sS", name="s0")
                s1 = psS.tile([128, 2 * QCH], F32, tag="sS", name="s1")
                for half, i in ((0, 2 * g), (1, 2 * g + 1)):
                    ks = slice(i * 128, (i + 1) * 128)
                    hs = slice(half * QCH, (half + 1) * QCH)
                    nc.tensor.matmul(
                        s0[:, hs], qk[1][0:64, ks], qk[0][0:64, qs],
                        start=True, stop=True, tile_position=(0, 0),
                    )
                    nc.tensor.matmul(
                        s1[:, hs], qk[1][64:128, ks], qk[0][64:128, qs],
                        start=True, stop=True, tile_position=(64, 0),
                    )
                pt = ptp.tile([128, 4 * QCH], BF16, tag="p01", name="pt01")
                nc.scalar.activation(
                    out=pt[:, 0 : 2 * QCH], in_=s0, func=EXP,
                    scale=0.125, bias=nbias[:, :],
                )
                nc.scalar.activation(
                    out=pt[:, 2 * QCH :], in_=s1, func=EXP,
                    scale=0.125, bias=nbias[:, :],
                )
                return pt

            def scores2(c, g):
                """h2 score tile (key tiles 2g | 2g+1 packed) + exp."""
                qs = slice(c * QCH, (c + 1) * QCH)
                i0, i1 = 2 * g, 2 * g + 1
                ksA = slice(i0 * 128, (i0 + 1) * 128)
                ksB = slice(i1 * 128, (i1 + 1) * 128)
                s2 = psS.tile([128, 2 * QCH], F32, tag="sS", name="s2")
                nc.tensor.matmul(
                    s2[:, 0:QCH], qk2d[0:64, ksA], qk[2][0:64, qs],
                    start=True, stop=True, tile_position=(0, 0),
                )
                nc.tensor.matmul(
                    s2[:, QCH:], qk[2][64:128, ksB], qk2d[64:128, qs],
                    start=True, stop=True, tile_position=(64, 0),
                )
                pt2 = ptp.tile([128, 2 * QCH], BF16, tag="p2", name="pt2")
                nc.scalar.activation(
                    out=pt2, in_=s2, func=EXP, scale=0.125, bias=nbias[:, :]
                )
                return pt2

            def attnv01(g, pt, pv0, pv1):
                for j in range(2):
                    nc.tensor.matmul(
                        pv0[0:65, :],
                        vtp[g][:, VW * j : VW * j + 65],
                        pt[:, j * QCH : (j + 1) * QCH],
                        start=(g == 0 and j == 0),
                        stop=(g == NG - 1 and j == 1),
                    )
                    nc.tensor.matmul(
                        pv1[0:65, :],
                        vtp[g][:, VW * j + 65 : VW * j + 130],
                        pt[:, 2 * QCH + j * QCH : 2 * QCH + (j + 1) * QCH],
                        start=(g == 0 and j == 0),
                        stop=(g == NG - 1 and j == 1),
                    )

            def attnv2(g, pt2, pv2):
                for j in range(2):
                    nc.tensor.matmul(
                        pv2[0:65, :],
                        vtp[g][:, VW * j + 130 : VW * j + 195],
                        pt2[:, j * QCH : (j + 1) * QCH],
                        start=(g == 0 and j == 0),
                        stop=(g == NG - 1 and j == 1),
                    )

            def vnat_mtile(i):
                """V projection for key tile i into vtp pair-interleaved."""
                g, j = i // 2, i % 2
                ps = cd_tile()
                for k in range(KT):
                    nc.tensor.matmul(
                        ps[:, 0:192],
                        xt[k][:, i * 128 : (i + 1) * 128],
                        wv_t[:, k * 192 : (k + 1) * 192],
                        start=(k == 0),
                        stop=(k == KT - 1),
                    )
                oap = (
                    vtp[g][:, VW * j : VW * j + 195]
                    .rearrange("p (h c) -> p h c", c=65)[:, :, 0:64]
                )
                iap = ps[:, 0:192].rearrange("p (h c) -> p h c", c=64)
                nc.vector.tensor_copy(out=oap, in_=iap)

            def divide01(c, pv0, pv1):
                qs = slice(c * QCH, (c + 1) * QCH)
                ovc = wk.tile([128, QCH], F32, tag="ov", name="ovc")
                nc.vector.tensor_copy(out=ovc[0:64, :], in_=pv0[0:64, :])
                nc.vector.tensor_copy(out=ovc[64:128, :], in_=pv1[0:64, :])
                sm = wk.tile([1, 2 * QCH], F32, tag="sm", name="sm")
                nc.vector.tensor_copy(out=sm[:, 0:QCH], in_=pv0[64:65, :])
                nc.vector.tensor_copy(out=sm[:, QCH:], in_=pv1[64:65, :])
                rw = wk.tile([128, 2 * QCH // 128], F32, tag="rw", name="rw")
                nc.sync.dma_start(out=rw, in_=sm)
                nc.vector.reciprocal(out=rw, in_=rw)
                rec = wk.tile([1, 2 * QCH], F32, tag="rc", name="rec")
                nc.sync.dma_start(out=rec, in_=rw)
                bc = wk.tile([128, QCH], F32, tag="bc", name="bc")
                nc.gpsimd.partition_broadcast(bc[0:64, :], rec[:, 0:QCH])
                tmpb = wk.tile([64, QCH], F32, tag="tb", name="tmpb")
                nc.gpsimd.partition_broadcast(tmpb, rec[:, QCH:])
                nc.sync.dma_start(out=bc[64:128, :], in_=tmpb)
                nc.vector.tensor_mul(out=ao01[:, qs], in0=ovc, in1=bc)

            def divide2(c, pv2, last=False):
                qs = slice(c * QCH, (c + 1) * QCH)
                ov2 = wk.tile([64, QCH], F32, tag="ov2", name="ov2")
                nc.vector.tensor_copy(out=ov2, in_=pv2[0:64, :])
                sm2 = wk.tile([1, QCH], F32, tag="sm2", name="sm2")
                nc.vector.tensor_copy(out=sm2, in_=pv2[64:65, :])
                rec2 = wk.tile([1, QCH], F32, tag="rc2", name="rec2")
                if last:
                    nc.vector.reciprocal(out=rec2, in_=sm2)
                else:
                    rw2 = wk.tile([128, QCH // 128], F32, tag="rw2", name="rw2")
                    nc.sync.dma_start(out=rw2, in_=sm2)
                    nc.vector.reciprocal(out=rw2, in_=rw2)
                    nc.sync.dma_start(out=rec2, in_=rw2)
                bc2 = wk.tile([64, QCH], F32, tag="bc2", name="bc2")
                nc.gpsimd.partition_broadcast(bc2, rec2)
                nc.vector.tensor_mul(out=ao2[0:64, qs], in0=ov2, in1=bc2)
                if not no_dup:
                    nc.sync.dma_start(out=ao2[64:128, qs], in_=ao2[0:64, qs])

            def proj_single(c, m, pool, tag):
                if no_proj:
                    return
                qs = slice(c * QCH, (c + 1) * QCH)
                ms = slice(m * 128, (m + 1) * 128)
                po = pool.tile([128, QCH], F32, tag=tag, name="po")
                nc.tensor.matmul(
                    po, wp01_t[:, ms], ao01[:, qs], start=True, stop=False
                )
                nc.tensor.matmul(
                    po, wp2_t[0:64, ms], ao2[0:64, qs],
                    start=False, stop=True, tile_position=(0, 0),
                )
                ot = otp.tile([128, QCH], BF16, tag="ot", name="ot")
                nc.vector.tensor_copy(out=ot, in_=po)
                nc.sync.dma_start(
                    out=outT[:, :].rearrange("(t p) n -> t p n", p=128)[m][:, qs],
                    in_=ot,
                )

            def proj_pair(c, me):
                if no_proj:
                    return
                """Projection m-tiles me, me+1: h01 K=128 each, h2 K=64
                quadrant-paired across banks C/D."""
                qs = slice(c * QCH, (c + 1) * QCH)
                msE = slice(me * 128, (me + 1) * 128)
                msO = slice((me + 1) * 128, (me + 2) * 128)
                poE = psC.tile([128, QCH], F32, tag="C", name="poE")
                poO = psD.tile([128, QCH], F32, tag="D", name="poO")
                nc.tensor.matmul(
                    poE, wp01_t[:, msE], ao01[:, qs], start=True, stop=False
                )
                nc.tensor.matmul(
                    poO, wp01_t[:, msO], ao01[:, qs], start=True, stop=False
                )
                nc.tensor.matmul(
                    poE, wp2_t[0:64, msE], ao2[0:64, qs],
                    start=False, stop=True, tile_position=(0, 0),
                )
                nc.tensor.matmul(
                    poO, wp2_t[64:128, msO], ao2[64:128, qs],
                    start=False, stop=True, tile_position=(64, 0),
                )
                for m, po in ((me, poE), (me + 1, poO)):
                    ot = otp.tile([128, QCH], BF16, tag="ot", name="ot")
                    nc.vector.tensor_copy(out=ot, in_=po)
                    nc.sync.dma_start(
                        out=outT[:, :].rearrange("(t p) n -> t p n", p=128)[m][
                            :, slice(c * QCH, (c + 1) * QCH)
                        ],
                        in_=ot,
                    )

            # ---- schedule ----
            # warm the PE clock gate with dummy matmuls while inputs stream in
            nc.vector.memset(wz, 0.0)
            pswz = psS.tile([128, 2 * QCH], F32, tag="sS", name="pswz")
            for r in range(10):
                nc.tensor.matmul(
                    pswz[:, 0:QCH], wz[:, 0:128], wz,
                    start=(r == 0), stop=(r == 9),
                )
            nc.vector.tensor_copy(out=warm0, in_=pswz[:, 0:8])

            # K h0/h1 cols 0:1024 and Q chunk 0, 512-col granularity on C/D
            qk_mtileH(1, 0)
            qk_mtileH(1, 1)
            qk_mtileH(0, 0)

            pt01 = {}
            pt2d = {}

            # ======== chunk 0: phase A carries the rest of the prologue ====
            pv0 = psA.tile([128, QCH], F32, tag="A", name="pv0")
            pv1 = psB.tile([128, QCH], F32, tag="B", name="pv1")
            extras = {
                1: [(0, 1)], 2: [(1, 2)], 3: [(1, 3)],
                4: [(2, 0), (2, 1)], 5: [(2, 2), (2, 3)],
            }
            for g in range(NG):
                pt01[g] = scores01(0, g)
                for m, h in extras.get(g, []):
                    qk_mtileH(m, h)
                if g == 5:
                    nc.sync.dma_start(out=qk2d[0:64, :], in_=qk[2][64:128, :])
                    nc.sync.dma_start(out=qk2d[64:128, :], in_=qk[2][0:64, :])
                if g >= 1:
                    vnat_mtile(2 * (g - 1))
                    vnat_mtile(2 * g - 1)
            vnat_mtile(2 * NG - 2)
            vnat_mtile(2 * NG - 1)

            # phase B(0): h2 scores + attn@V(h0,h1) + attn@V(h2) lag-2
            pv2p = psC.tile([128, QCH], F32, tag="C", name="pv2")
            for g in range(NG):
                pt2d[g] = scores2(0, g)
                attnv01(g, pt01.pop(g), pv0, pv1)
                if g >= 2:
                    attnv2(g - 2, pt2d.pop(g - 2), pv2p)
            divide01(0, pv0, pv1)
            if no_spill:
                attnv2(6, pt2d.pop(6), pv2p)
                attnv2(7, pt2d.pop(7), pv2p)
                divide2(0, pv2p)
            # (else: attn@V(h2) groups 6,7 + divide2(0) spill into phase A(1))

            # ======== chunks 1, 2 ========
            for c in (1, 2):
                pv0 = psA.tile([128, QCH], F32, tag="A", name="pv0")
                pv1 = psB.tile([128, QCH], F32, tag="B", name="pv1")
                for g in range(NG):
                    pt01[g] = scores01(c, g)
                    if c == 1 and g < 2 and not no_spill:
                        attnv2(6 + g, pt2d.pop(6 + g), pv2p)
                        if g == 1:
                            divide2(0, pv2p)
                    if c == 1 and g == 3:
                        qk_mtileH(0, 2)
                    if c == 2 and g == 4:
                        qk_mtileH(0, 3)
                    if c == 2 and g in (1, 3, 5):
                        proj_pair(0, g - 1)
                    if g >= 2:
                        attnv01(g - 2, pt01.pop(g - 2), pv0, pv1)
                attnv01(NG - 2, pt01.pop(NG - 2), pv0, pv1)
                attnv01(NG - 1, pt01.pop(NG - 1), pv0, pv1)
                divide01(c, pv0, pv1)

                pv2 = psC.tile([128, QCH], F32, tag="C", name="pv2")
                for g in range(NG):
                    pt2d[g] = scores2(c, g)
                    if g >= 2:
                        attnv2(g - 2, pt2d.pop(g - 2), pv2)
                attnv2(NG - 2, pt2d.pop(NG - 2), pv2)
                attnv2(NG - 1, pt2d.pop(NG - 1), pv2)
                divide2(c, pv2)

            # ======== chunk 3: h0/h1 first (+proj(1)); h2 last (+proj(2));
            # tail is attnv2 end + short divide2 + proj(3) ========
            pv0 = psA.tile([128, QCH], F32, tag="A", name="pv0")
            pv1 = psB.tile([128, QCH], F32, tag="B", name="pv1")
            for g in range(NG):
                pt01[g] = scores01(3, g)
                if g in (1, 3, 5):
                    proj_pair(1, g - 1)
                if g >= 2:
                    attnv01(g - 2, pt01.pop(g - 2), pv0, pv1)
            attnv01(NG - 2, pt01.pop(NG - 2), pv0, pv1)
            attnv01(NG - 1, pt01.pop(NG - 1), pv0, pv1)
            divide01(3, pv0, pv1)

            pv2 = psC.tile([128, QCH], F32, tag="C", name="pv2")
            for g in range(NG):
                pt2d[g] = scores2(3, g)
                if g >= 2:
                    attnv2(g - 2, pt2d.pop(g - 2), pv2)
                if g < KT:
                    proj_single(2, g, psD, "D")
            attnv2(NG - 2, pt2d.pop(NG - 2), pv2)
            attnv2(NG - 1, pt2d.pop(NG - 1), pv2)

            # epilogue: h01 matmuls of proj(3) pairs 0,1 run during the
            # divide2(3) latency chain (idle psS banks); pair 2 follows on
            # C/D once pv2 has drained.
            qs3 = slice(3 * QCH, 4 * QCH)
            epi = []
            for me in (0, 2):
                t = psS.tile([128, 2 * QCH], F32, tag="sS", name=f"poP{me}")
                epi.append((me, t[:, 0:QCH], t[:, QCH:]))
            for me, poE, poO in epi:
                nc.tensor.matmul(
                    poE, wp01_t[:, me * 128 : (me + 1) * 128], ao01[:, qs3],
                    start=True, stop=False,
                )
                nc.tensor.matmul(
                    poO, wp01_t[:, (me + 1) * 128 : (me + 2) * 128], ao01[:, qs3],
                    start=True, stop=False,
                )
            divide2(3, pv2, last=True)
            poE2 = psC.tile([128, QCH], F32, tag="C", name="poE2")
            poO2 = psD.tile([128, QCH], F32, tag="D", name="poO2")
            nc.tensor.matmul(
                poE2, wp01_t[:, 4 * 128 : 5 * 128], ao01[:, qs3],
                start=True, stop=False,
            )
            nc.tensor.matmul(
                poO2, wp01_t[:, 5 * 128 : 6 * 128], ao01[:, qs3],
                start=True, stop=False,
            )
            epi.append((4, poE2, poO2))
            for me, poE, poO in epi:
                nc.tensor.matmul(
                    poE, wp2_t[0:64, me * 128 : (me + 1) * 128], ao2[0:64, qs3],
                    start=False, stop=True, tile_position=(0, 0),
                )
                nc.tensor.matmul(
                    poO, wp2_t[64:128, (me + 1) * 128 : (me + 2) * 128],
                    ao2[64:128, qs3],
                    start=False, stop=True, tile_position=(64, 0),
                )
                for m, po in ((me, poE), (me + 1, poO)):
                    ot = otp.tile([128, QCH], BF16, tag="ot", name="ot")
                    nc.vector.tensor_copy(out=ot, in_=po)
                    nc.sync.dma_start(
                        out=outT[:, :].rearrange("(t p) n -> t p n", p=128)[m][
                            :, qs3
                        ],
                        in_=ot,
                    )

            if debug:
                for m in range(3):
                    nc.sync.dma_start(out=dbg_qk[m], in_=qk[m][:, :])
                nc.sync.dma_start(out=dbg_qk[3], in_=qk2d[:, :])
                for g in range(NG):
                    nc.sync.dma_start(out=dbg_vt[g], in_=vtp[g][:, :])
                nc.sync.dma_start(out=dbg_ao[0], in_=ao01[:, :])
                nc.sync.dma_start(out=dbg_ao[1], in_=ao2[:, :])

    nc.compile()
    return nc


def _get_nc():
    if "nc" not in _CACHED:
        _CACHED["nc"] = _build()
    return _CACHED["nc"]


def _shard_inputs(x, w_qkv, w_proj):
    """Build the 8 per-core input maps."""
    import ml_dtypes

    bf = ml_dtypes.bfloat16
    in_maps = []
    for core in range(N_CORES):
        b = core // 4
        h0 = 3 * (core % 4)
        heads = [h0, h0 + 1, h0 + 2]
        xTc = np.ascontiguousarray(x[b].T).astype(bf)
        wq = [w_qkv[:, h * HD : (h + 1) * HD] for h in heads]
        wk_ = [w_qkv[:, EMBED + h * HD : EMBED + (h + 1) * HD] for h in heads]
        wv_ = [
            w_qkv[:, 2 * EMBED + h * HD : 2 * EMBED + (h + 1) * HD] for h in heads
        ]
        wqk = np.concatenate(
            [wq[0], wq[1], wk_[0], wk_[1], wq[2], wk_[2]], axis=1
        ).astype(bf)
        wvc = np.concatenate([wv_[0], wv_[1], wv_[2]], axis=1).astype(bf)
        wp01 = np.ascontiguousarray(
            w_proj[heads[0] * HD : (heads[0] + 2) * HD, :]
        ).astype(bf)
        wp2row = w_proj[heads[2] * HD : (heads[2] + 1) * HD, :]
        wp2 = np.ascontiguousarray(
            np.concatenate([wp2row, wp2row], axis=0)
        ).astype(bf)
        in_maps.append(
            {
                "xT": xTc,
                "wqk": np.ascontiguousarray(wqk),
                "wv": np.ascontiguousarray(wvc),
                "wp01": wp01,
                "wp2": wp2,
            }
        )
    return in_maps


def kernel(x, w_qkv, w_proj, _trace=False):
    from concourse.bass_utils import run_bass_kernel_spmd

    x = np.asarray(x, dtype=np.float32)
    w_qkv = np.asarray(w_qkv, dtype=np.float32)
    w_proj = np.asarray(w_proj, dtype=np.float32)

    nc = _get_nc()
    in_maps = _shard_inputs(x, w_qkv, w_proj)
    res = run_bass_kernel_spmd(
        nc, in_maps, core_ids=list(range(N_CORES)), trace=_trace
    )
    _CACHED["last_results"] = res

    out = np.empty((2, N_SEQ, EMBED), dtype=np.float32)
    for b in range(2):
        acc = res.results[4 * b]["outT"].astype(np.float32).copy()
        for g in range(1, 4):
            acc += res.results[4 * b + g]["outT"]
        out[b] = acc.T
    return out


# revision 35
# speedup vs baseline: 1.0193x; 1.0193x over previous
"""Multi-head self-attention Trainium2 kernel.

Problem: x[2, 2048, 768] -> MHSA (12 heads, head_dim 64) -> out[2, 2048, 768].

Sharding over 8 NeuronCores: core c handles batch c//4 and heads
[3*(c%4), 3*(c%4)+3). Each core computes its 3 heads' attention and a
row-split partial of the output projection over its 192 channels (bf16
partials); the host sums the 4 partials per batch and transposes.

The ScalarE exp stream (96 activations, ~110us) is the bottleneck; the
kernel is a just-in-time pipeline that keeps it saturated:

- per chunk c: phase A emits h0/h1 score groups (4 quadrant-paired matmuls
  -> 2 exps into one [128,2048] pt tile) and runs attn@V(h0,h1) two groups
  behind (lag-2 so attn@V never blocks the in-order TensorE queue on an
  exp it consumes); phase B emits h2 score groups, runs attn@V(h2) lag-2
  and the previous chunk's projection.
- chunk 0's phase A slots carry the QKV projections (512-col granularity
  on the idle proj PSUM banks) and V-natural tiles; DMA issue order is
  arranged so no early matmul head-of-line-blocks the queue on a late DMA.
- chunk 3 runs h0/h1 first so only the short h2 divide chain and the
  final projection (pre-issued h01 matmuls, pairs on idle score banks)
  trail the last exp.
- softmax row-sums ride attn@V as an appended ones-column; exp computes
  exp(s/8 - 2) (shift-invariant); the divide is a 128-lane reciprocal
  (reshape DMAs) + GPSIMD partition broadcast (dest must be partition 0;
  upper half via DMA hop).
- projection packs heads 0,1 into one K=128 matmul (ao stacked [128,seq])
  and quadrant-pairs the two K=64 h2 matmuls across banks.
- PSUM: 4 banks score ring (2x[128,1024]) + 4 single-bank accumulators
  (pv0, pv1, pv2/proj-even, proj-odd).
- a dummy-matmul burst at t=0 warms the PE HAM clock gate; a warmup
  activation preloads the exp table during the input DMA phase.
"""

import sys

sys.path.insert(0, "/opt/trn_rl_repo")

import numpy as np

EMBED = 768
N_SEQ = 2048
HD = 64
N_CORES = 8
KT = EMBED // 128  # 6 contraction tiles
MT = N_SEQ // 128  # 16 key tiles
QCH = 512
NCH = N_SEQ // QCH  # 4 chunks
NG = MT // 2  # 8 groups of 2 key tiles
VW = 208  # per-j V block width in vtp (3*65 ones-interleaved + pad)

_CACHED = {}


def _build(debug=False, no_proj=False, no_dup=False, no_spill=False, no_c3r=False):
    from concourse import bacc
    import concourse.tile as tile
    import concourse.mybir as mybir

    F32 = mybir.dt.float32
    BF16 = mybir.dt.bfloat16
    EXP = mybir.ActivationFunctionType.Exp

    nc = bacc.Bacc()
    xT = nc.declare_dram_parameter("xT", [EMBED, N_SEQ], BF16, isOutput=False)
    # Q/K weights in 3 m-tiles of 128 cols: [Qh0|Qh1], [Kh0|Kh1], [Qh2|Kh2]
    wqk = nc.declare_dram_parameter("wqk", [EMBED, 384], BF16, isOutput=False)
    wv = nc.declare_dram_parameter("wv", [EMBED, 192], BF16, isOutput=False)
    wp01 = nc.declare_dram_parameter("wp01", [128, EMBED], BF16, isOutput=False)
    wp2 = nc.declare_dram_parameter("wp2", [128, EMBED], BF16, isOutput=False)
    outT = nc.declare_dram_parameter("outT", [EMBED, N_SEQ], BF16, isOutput=True)
    if debug:
        dbg_qk = nc.declare_dram_parameter("dbg_qk", [4, 128, N_SEQ], BF16, isOutput=True)
        dbg_vt = nc.declare_dram_parameter("dbg_vt", [NG, 128, 2 * VW], BF16, isOutput=True)
        dbg_ao = nc.declare_dram_parameter("dbg_ao", [2, 128, N_SEQ], BF16, isOutput=True)

    with tile.TileContext(nc) as tc:
        with (
            tc.tile_pool(name="persist", bufs=1) as pp,
            tc.tile_pool(name="pt", bufs=10) as ptp,
            tc.tile_pool(name="work", bufs=2) as wk,
            tc.tile_pool(name="otp", bufs=6) as otp,
            tc.tile_pool(name="psS", bufs=2, space="PSUM") as psS,
            tc.tile_pool(name="psA", bufs=1, space="PSUM") as psA,
            tc.tile_pool(name="psB", bufs=1, space="PSUM") as psB,
            tc.tile_pool(name="psC", bufs=1, space="PSUM") as psC,
            tc.tile_pool(name="psD", bufs=1, space="PSUM") as psD,
        ):
            qk = [
                pp.tile([128, N_SEQ], BF16, tag=f"qk{m}", name=f"qk{m}")
                for m in range(3)
            ]
            qk2d = pp.tile([128, N_SEQ], BF16, tag="qk2d")
            vtp = [
                pp.tile([128, 2 * VW], BF16, tag=f"vt{g}", name=f"vt{g}")
                for g in range(NG)
            ]
            xt = [
                pp.tile([128, N_SEQ], BF16, tag=f"xt{k}", name=f"xt{k}")
                for k in range(KT)
            ]
            wqk_t = [
                pp.tile([128, 384], BF16, tag=f"wqk{k}", name=f"wqkt{k}")
                for k in range(KT)
            ]
            wv_t = pp.tile([128, KT * 192], BF16, tag="wv")
            wp01_t = pp.tile([128, EMBED], BF16, tag="wp01")
            wp2_t = pp.tile([128, EMBED], BF16, tag="wp2")
            ao01 = pp.tile([128, N_SEQ], BF16, tag="ao01")
            ao2 = pp.tile([128, N_SEQ], BF16, tag="ao2")
            warm0 = pp.tile([128, 8], F32, tag="warm0")
            wz = pp.tile([128, QCH], BF16, tag="wz")
            warm1 = pp.tile([128, 8], BF16, tag="warm1")
            nbias = pp.tile([128, 1], F32, tag="nbias")

            xT_ap = xT[:, :].rearrange("(t p) n -> t p n", p=128)
            wqk_ap = wqk[:, :].rearrange("(t p) n -> t p n", p=128)
            wv_ap = wv[:, :].rearrange("(t p) n -> t p n", p=128)

            # ---- DMA issue order == Sync queue order ----
            for k in range(KT):
                nc.sync.dma_start(out=wqk_t[k], in_=wqk_ap[k])
                nc.sync.dma_start(
                    out=xt[k][:, 0 : 2 * QCH], in_=xT_ap[k][:, 0 : 2 * QCH]
                )
            for k in range(KT):
                nc.sync.dma_start(
                    out=wv_t[:, k * 192 : (k + 1) * 192], in_=wv_ap[k]
                )
            for k in range(KT):
                nc.sync.dma_start(
                    out=xt[k][:, 2 * QCH :], in_=xT_ap[k][:, 2 * QCH :]
                )
            nc.sync.dma_start(out=wp01_t, in_=wp01[:, :])
            nc.sync.dma_start(out=wp2_t, in_=wp2[:, :])

            # ones columns of vtp via DVE (keep the Sync queue clear)
            for g in range(NG):
                for j in range(2):
                    oap = (
                        vtp[g][:, VW * j : VW * j + 195]
                        .rearrange("p (h c) -> p h c", c=65)[:, :, 64]
                    )
                    nc.vector.memset(oap, 1.0)
            # warmup: pull the exp table load off the critical path
            nc.vector.memset(warm0, 0.0)
            nc.vector.memset(nbias, -2.0)
            nc.scalar.activation(out=warm1, in_=warm0, func=EXP, scale=0.125)

            # ---- building blocks ----
            cdflip = [0]

            def cd_tile():
                pool, tag = (psC, "C") if cdflip[0] == 0 else (psD, "D")
                cdflip[0] ^= 1
                return pool.tile([128, QCH], F32, tag=tag, name="pcd")

            def qk_mtileH(m, h):
                """QKV projection m-tile over 512 cols [h*512, +512)."""
                ps = cd_tile()
                lo = h * QCH
                for k in range(KT):
                    nc.tensor.matmul(
                        ps,
                        wqk_t[k][:, m * 128 : (m + 1) * 128],
                        xt[k][:, lo : lo + QCH],
                        start=(k == 0),
                        stop=(k == KT - 1),
                    )
                nc.vector.tensor_copy(out=qk[m][:, lo : lo + QCH], in_=ps)

            def scores01(c, g):
                """h0/h1 score tiles for key tiles 2g,2g+1; exp into one pt."""
                qs = slice(c * QCH, (c + 1) * QCH)
                s0 = psS.tile([128, 2 * QCH], F32, tag="sS", name="s0")
                s1 = psS.tile([128, 2 * QCH], F32, tag="sS", name="s1")
                for half, i in ((0, 2 * g), (1, 2 * g + 1)):
                    ks = slice(i * 128, (i + 1) * 128)
                    hs = slice(half * QCH, (half + 1) * QCH)
                    nc.tensor.matmul(
                        s0[:, hs], qk[1][0:64, ks], qk[0][0:64, qs],
                        start=True, stop=True, tile_position=(0, 0),
                    )
                    nc.tensor.matmul(
                        s1[:, hs], qk[1][64:128, ks], qk[0][64:128, qs],
                        start=True, stop=True, tile_position=(64, 0),
                    )
                pt = ptp.tile([128, 4 * QCH], BF16, tag="p01", name="pt01")
                nc.scalar.activation(
                    out=pt[:, 0 : 2 * QCH], in_=s0, func=EXP,
                    scale=0.125, bias=nbias[:, :],
                )
                nc.scalar.activation(
                    out=pt[:, 2 * QCH :], in_=s1, func=EXP,
                    scale=0.125, bias=nbias[:, :],
                )
                return pt

            def scores2(c, g):
                """h2 score tile (key tiles 2g | 2g+1 packed) + exp."""
                qs = slice(c * QCH, (c + 1) * QCH)
                i0, i1 = 2 * g, 2 * g + 1
                ksA = slice(i0 * 128, (i0 + 1) * 128)
                ksB = slice(i1 * 128, (i1 + 1) * 128)
                s2 = psS.tile([128, 2 * QCH], F32, tag="sS", name="s2")
                nc.tensor.matmul(
                    s2[:, 0:QCH], qk2d[0:64, ksA], qk[2][0:64, qs],
                    start=True, stop=True, tile_position=(0, 0),
                )
                nc.tensor.matmul(
                    s2[:, QCH:], qk[2][64:128, ksB], qk2d[64:128, qs],
                    start=True, stop=True, tile_position=(64, 0),
                )
                pt2 = ptp.tile([128, 2 * QCH], BF16, tag="p2", name="pt2")
                nc.scalar.activation(
                    out=pt2, in_=s2, func=EXP, scale=0.125, bias=nbias[:, :]
                )
                return pt2

            def attnv01(g, pt, pv0, pv1):
                for j in range(2):
                    nc.tensor.matmul(
                        pv0[0:65, :],
                        vtp[g][:, VW * j : VW * j + 65],
                        pt[:, j * QCH : (j + 1) * QCH],
                        start=(g == 0 and j == 0),
                        stop=(g == NG - 1 and j == 1),
                    )
                    nc.tensor.matmul(
                        pv1[0:65, :],
                        vtp[g][:, VW * j + 65 : VW * j + 130],
                        pt[:, 2 * QCH + j * QCH : 2 * QCH + (j + 1) * QCH],
                        start=(g == 0 and j == 0),
                        stop=(g == NG - 1 and j == 1),
                    )

            def attnv2(g, pt2, pv2):
                for j in range(2):
                    nc.tensor.matmul(
                        pv2[0:65, :],
                        vtp[g][:, VW * j + 130 : VW * j + 195],
                        pt2[:, j * QCH : (j + 1) * QCH],
                        start=(g == 0 and j == 0),
                        stop=(g == NG - 1 and j == 1),
                    )

            def vnat_mtile(i):
                """V projection for key tile i into vtp pair-interleaved."""
                g, j = i // 2, i % 2
                ps = cd_tile()
                for k in range(KT):
                    nc.tensor.matmul(
                        ps[:, 0:192],
                        xt[k][:, i * 128 : (i + 1) * 128],
                        wv_t[:, k * 192 : (k + 1) * 192],
                        start=(k == 0),
                        stop=(k == KT - 1),
                    )
                oap = (
                    vtp[g][:, VW * j : VW * j + 195]
                    .rearrange("p (h c) -> p h c", c=65)[:, :, 0:64]
                )
                iap = ps[:, 0:192].rearrange("p (h c) -> p h c", c=64)
                nc.vector.tensor_copy(out=oap, in_=iap)

            def divide01(c, pv0, pv1):
                qs = slice(c * QCH, (c + 1) * QCH)
                ovc = wk.tile([128, QCH], F32, tag="ov", name="ovc")
                nc.vector.tensor_copy(out=ovc[0:64, :], in_=pv0[0:64, :])
                nc.vector.tensor_copy(out=ovc[64:128, :], in_=pv1[0:64, :])
                sm = wk.tile([1, 2 * QCH], F32, tag="sm", name="sm")
                nc.vector.tensor_copy(out=sm[:, 0:QCH], in_=pv0[64:65, :])
                nc.vector.tensor_copy(out=sm[:, QCH:], in_=pv1[64:65, :])
                rw = wk.tile([128, 2 * QCH // 128], F32, tag="rw", name="rw")
                nc.sync.dma_start(out=rw, in_=sm)
                nc.vector.reciprocal(out=rw, in_=rw)
                rec = wk.tile([1, 2 * QCH], F32, tag="rc", name="rec")
                nc.sync.dma_start(out=rec, in_=rw)
                bc = wk.tile([128, QCH], F32, tag="bc", name="bc")
                nc.gpsimd.partition_broadcast(bc[0:64, :], rec[:, 0:QCH])
                tmpb = wk.tile([64, QCH], F32, tag="tb", name="tmpb")
                nc.gpsimd.partition_broadcast(tmpb, rec[:, QCH:])
                nc.sync.dma_start(out=bc[64:128, :], in_=tmpb)
                nc.vector.tensor_mul(out=ao01[:, qs], in0=ovc, in1=bc)

            def divide2(c, pv2):
                qs = slice(c * QCH, (c + 1) * QCH)
                ov2 = wk.tile([64, QCH], F32, tag="ov2", name="ov2")
                nc.vector.tensor_copy(out=ov2, in_=pv2[0:64, :])
                sm2 = wk.tile([1, QCH], F32, tag="sm2", name="sm2")
                nc.vector.tensor_copy(out=sm2, in_=pv2[64:65, :])
                rw2 = wk.tile([128, QCH // 128], F32, tag="rw2", name="rw2")
                nc.sync.dma_start(out=rw2, in_=sm2)
                nc.vector.reciprocal(out=rw2, in_=rw2)
                rec2 = wk.tile([1, QCH], F32, tag="rc2", name="rec2")
                nc.sync.dma_start(out=rec2, in_=rw2)
                bc2 = wk.tile([64, QCH], F32, tag="bc2", name="bc2")
                nc.gpsimd.partition_broadcast(bc2, rec2)
                nc.vector.tensor_mul(out=ao2[0:64, qs], in0=ov2, in1=bc2)
                if not no_dup:
                    nc.sync.dma_start(out=ao2[64:128, qs], in_=ao2[0:64, qs])

            def proj_single(c, m, pool, tag):
                if no_proj:
                    return
                qs = slice(c * QCH, (c + 1) * QCH)
                ms = slice(m * 128, (m + 1) * 128)
                po = pool.tile([128, QCH], F32, tag=tag, name="po")
                nc.tensor.matmul(
                    po, wp01_t[:, ms], ao01[:, qs], start=True, stop=False
                )
                nc.tensor.matmul(
                    po, wp2_t[0:64, ms], ao2[0:64, qs],
                    start=False, stop=True, tile_position=(0, 0),
                )
                ot = otp.tile([128, QCH], BF16, tag="ot", name="ot")
                nc.vector.tensor_copy(out=ot, in_=po)
                nc.sync.dma_start(
                    out=outT[:, :].rearrange("(t p) n -> t p n", p=128)[m][:, qs],
                    in_=ot,
                )

            def proj_pair(c, me):
                if no_proj:
                    return
                """Projection m-tiles me, me+1: h01 K=128 each, h2 K=64
                quadrant-paired across banks C/D."""
                qs = slice(c * QCH, (c + 1) * QCH)
                msE = slice(me * 128, (me + 1) * 128)
                msO = slice((me + 1) * 128, (me + 2) * 128)
                poE = psC.tile([128, QCH], F32, tag="C", name="poE")
                poO = psD.tile([128, QCH], F32, tag="D", name="poO")
                nc.tensor.matmul(
                    poE, wp01_t[:, msE], ao01[:, qs], start=True, stop=False
                )
                nc.tensor.matmul(
                    poO, wp01_t[:, msO], ao01[:, qs], start=True, stop=False
                )
                nc.tensor.matmul(
                    poE, wp2_t[0:64, msE], ao2[0:64, qs],
                    start=False, stop=True, tile_position=(0, 0),
                )
                nc.tensor.matmul(
                    poO, wp2_t[64:128, msO], ao2[64:128, qs],
                    start=False, stop=True, tile_position=(64, 0),
                )
                for m, po in ((me, poE), (me + 1, poO)):
                    ot = otp.tile([128, QCH], BF16, tag="ot", name="ot")
                    nc.vector.tensor_copy(out=ot, in_=po)
                    nc.sync.dma_start(
                        out=outT[:, :].rearrange("(t p) n -> t p n", p=128)[m][
                            :, slice(c * QCH, (c + 1) * QCH)
                        ],
                        in_=ot,
                    )

            # ---- schedule ----
            # warm the PE clock gate with dummy matmuls while inputs stream in
            nc.vector.memset(wz, 0.0)
            pswz = psS.tile([128, 2 * QCH], F32, tag="sS", name="pswz")
            for r in range(10):
                nc.tensor.matmul(
                    pswz[:, 0:QCH], wz[:, 0:128], wz,
                    start=(r == 0), stop=(r == 9),
                )
            nc.vector.tensor_copy(out=warm0, in_=pswz[:, 0:8])

            # K h0/h1 cols 0:1024 and Q chunk 0, 512-col granularity on C/D
            qk_mtileH(1, 0)
            qk_mtileH(1, 1)
            qk_mtileH(0, 0)

            pt01 = {}
            pt2d = {}

            # ======== chunk 0: phase A carries the rest of the prologue ====
            pv0 = psA.tile([128, QCH], F32, tag="A", name="pv0")
            pv1 = psB.tile([128, QCH], F32, tag="B", name="pv1")
            extras = {
                1: [(0, 1)], 2: [(1, 2)], 3: [(1, 3)],
                4: [(2, 0), (2, 1)], 5: [(2, 2), (2, 3)],
            }
            for g in range(NG):
                pt01[g] = scores01(0, g)
                for m, h in extras.get(g, []):
                    qk_mtileH(m, h)
                if g == 5:
                    nc.sync.dma_start(out=qk2d[0:64, :], in_=qk[2][64:128, :])
                    nc.sync.dma_start(out=qk2d[64:128, :], in_=qk[2][0:64, :])
                if g >= 1:
                    vnat_mtile(2 * (g - 1))
                    vnat_mtile(2 * g - 1)
            vnat_mtile(2 * NG - 2)
            vnat_mtile(2 * NG - 1)

            # phase B(0): h2 scores + attn@V(h0,h1) + attn@V(h2) lag-2
            pv2p = psC.tile([128, QCH], F32, tag="C", name="pv2")
            for g in range(NG):
                pt2d[g] = scores2(0, g)
                attnv01(g, pt01.pop(g), pv0, pv1)
                if g >= 2:
                    attnv2(g - 2, pt2d.pop(g - 2), pv2p)
            divide01(0, pv0, pv1)
            if no_spill:
                attnv2(6, pt2d.pop(6), pv2p)
                attnv2(7, pt2d.pop(7), pv2p)
                divide2(0, pv2p)
            # (else: attn@V(h2) groups 6,7 + divide2(0) spill into phase A(1))

            # ======== chunks 1, 2 ========
            for c in (1, 2):
                pv0 = psA.tile([128, QCH], F32, tag="A", name="pv0")
                pv1 = psB.tile([128, QCH], F32, tag="B", name="pv1")
                for g in range(NG):
                    pt01[g] = scores01(c, g)
                    if c == 1 and g < 2 and not no_spill:
                        attnv2(6 + g, pt2d.pop(6 + g), pv2p)
                        if g == 1:
                            divide2(0, pv2p)
                    if c == 1 and g == 3:
                        qk_mtileH(0, 2)
                    if c == 2 and g == 4:
                        qk_mtileH(0, 3)
                    if c == 2 and g in (1, 3, 5):
                        proj_pair(0, g - 1)
                    if g >= 2:
                        attnv01(g - 2, pt01.pop(g - 2), pv0, pv1)
                attnv01(NG - 2, pt01.pop(NG - 2), pv0, pv1)
                attnv01(NG - 1, pt01.pop(NG - 1), pv0, pv1)
                divide01(c, pv0, pv1)

                pv2 = psC.tile([128, QCH], F32, tag="C", name="pv2")
                for g in range(NG):
                    pt2d[g] = scores2(c, g)
                    if g >= 2:
                        attnv2(g - 2, pt2d.pop(g - 2), pv2)
                attnv2(NG - 2, pt2d.pop(NG - 2), pv2)
                attnv2(NG - 1, pt2d.pop(NG - 1), pv2)
                divide2(c, pv2)

            # ======== chunk 3: h0/h1 first (+proj(1)); h2 last (+proj(2));
            # tail is attnv2 end + short divide2 + proj(3) ========
            pv0 = psA.tile([128, QCH], F32, tag="A", name="pv0")
            pv1 = psB.tile([128, QCH], F32, tag="B", name="pv1")
            for g in range(NG):
                pt01[g] = scores01(3, g)
                if g in (1, 3, 5):
                    proj_pair(1, g - 1)
                if g >= 2:
                    attnv01(g - 2, pt01.pop(g - 2), pv0, pv1)
            attnv01(NG - 2, pt01.pop(NG - 2), pv0, pv1)
            attnv01(NG - 1, pt01.pop(NG - 1), pv0, pv1)
            divide01(3, pv0, pv1)

            pv2 = psC.tile([128, QCH], F32, tag="C", name="pv2")
            for g in range(NG):
                pt2d[g] = scores2(3, g)
                if g >= 2:
                    attnv2(g - 2, pt2d.pop(g - 2), pv2)
                if g < KT:
                    proj_single(2, g, psD, "D")
            attnv2(NG - 2, pt2d.pop(NG - 2), pv2)
            attnv2(NG - 1, pt2d.pop(NG - 1), pv2)

            # epilogue: h01 matmuls of proj(3) pairs 0,1 run during the
            # divide2(3) latency chain (idle psS banks); pair 2 follows on
            # C/D once pv2 has drained.
            qs3 = slice(3 * QCH, 4 * QCH)
            epi = []
            for me in (0, 2):
                t = psS.tile([128, 2 * QCH], F32, tag="sS", name=f"poP{me}")
                epi.append((me, t[:, 0:QCH], t[:, QCH:]))
            for me, poE, poO in epi:
                nc.tensor.matmul(
                    poE, wp01_t[:, me * 128 : (me + 1) * 128], ao01[:, qs3],
                    start=True, stop=False,
                )
                nc.tensor.matmul(
                    poO, wp01_t[:, (me + 1) * 128 : (me + 2) * 128], ao01[:, qs3],
                    start=True, stop=False,
                )
            divide2(3, pv2)
            poE2 = psC.tile([128, QCH], F32, tag="C", name="poE2")
            poO2 = psD.tile([128, QCH], F32, tag="D", name="poO2")
            nc.tensor.matmul(
                poE2, wp01_t[:, 4 * 128 : 5 * 128], ao01[:, qs3],
                start=True, stop=False,
            )
            nc.tensor.matmul(
                poO2, wp01_t[:, 5 * 128 : 6 * 128], ao01[:, qs3],
                start=True, stop=False,
            )
            epi.append((4, poE2, poO2))
            # keep the PE clock gate warm across the divide2 latency chain so
            # the h2 projection matmuls below run at full clock
            pswt = psA.tile([128, QCH], F32, tag="A", name="pswt")
            for r in range(16):
                nc.tensor.matmul(
                    pswt, wz[:, 0:128], wz,
                    start=(r == 0), stop=(r == 15),
                )
            nc.vector.tensor_copy(out=warm0, in_=pswt[:, 0:8])
            for me, poE, poO in epi:
                nc.tensor.matmul(
                    poE, wp2_t[0:64, me * 128 : (me + 1) * 128], ao2[0:64, qs3],
                    start=False, stop=True, tile_position=(0, 0),
                )
                nc.tensor.matmul(
                    poO, wp2_t[64:128, (me + 1) * 128 : (me + 2) * 128],
                    ao2[64:128, qs3],
                    start=False, stop=True, tile_position=(64, 0),
                )
                for m, po in ((me, poE), (me + 1, poO)):
                    ot = otp.tile([128, QCH], BF16, tag="ot", name="ot")
                    nc.vector.tensor_copy(out=ot, in_=po)
                    nc.sync.dma_start(
                        out=outT[:, :].rearrange("(t p) n -> t p n", p=128)[m][
                            :, qs3
                        ],
                        in_=ot,
                    )

            if debug:
                for m in range(3):
                    nc.sync.dma_start(out=dbg_qk[m], in_=qk[m][:, :])
                nc.sync.dma_start(out=dbg_qk[3], in_=qk2d[:, :])
                for g in range(NG):
                    nc.sync.dma_start(out=dbg_vt[g], in_=vtp[g][:, :])
                nc.sync.dma_start(out=dbg_ao[0], in_=ao01[:, :])
                nc.sync.dma_start(out=dbg_ao[1], in_=ao2[:, :])

    nc.compile()
    return nc


def _get_nc():
    if "nc" not in _CACHED:
        _CACHED["nc"] = _build()
    return _CACHED["nc"]


def _shard_inputs(x, w_qkv, w_proj):
    """Build the 8 per-core input maps."""
    import ml_dtypes

    bf = ml_dtypes.bfloat16
    in_maps = []
    for core in range(N_CORES):
        b = core // 4
        h0 = 3 * (core % 4)
        heads = [h0, h0 + 1, h0 + 2]
        xTc = np.ascontiguousarray(x[b].T).astype(bf)
        wq = [w_qkv[:, h * HD : (h + 1) * HD] for h in heads]
        wk_ = [w_qkv[:, EMBED + h * HD : EMBED + (h + 1) * HD] for h in heads]
        wv_ = [
            w_qkv[:, 2 * EMBED + h * HD : 2 * EMBED + (h + 1) * HD] for h in heads
        ]
        wqk = np.concatenate(
            [wq[0], wq[1], wk_[0], wk_[1], wq[2], wk_[2]], axis=1
        ).astype(bf)
        wvc = np.concatenate([wv_[0], wv_[1], wv_[2]], axis=1).astype(bf)
        wp01 = np.ascontiguousarray(
            w_proj[heads[0] * HD : (heads[0] + 2) * HD, :]
        ).astype(bf)
        wp2row = w_proj[heads[2] * HD : (heads[2] + 1) * HD, :]
        wp2 = np.ascontiguousarray(
            np.concatenate([wp2row, wp2row], axis=0)
        ).astype(bf)
        in_maps.append(
            {
                "xT": xTc,
                "wqk": np.ascontiguousarray(wqk),
                "wv": np.ascontiguousarray(wvc),
                "wp01": wp01,
                "wp2": wp2,
            }
        )
    return in_maps


def kernel(x, w_qkv, w_proj, _trace=False):
    from concourse.bass_utils import run_bass_kernel_spmd

    x = np.asarray(x, dtype=np.float32)
    w_qkv = np.asarray(w_qkv, dtype=np.float32)
    w_proj = np.asarray(w_proj, dtype=np.float32)

    nc = _get_nc()
    in_maps = _shard_inputs(x, w_qkv, w_proj)
    res = run_bass_kernel_spmd(
        nc, in_maps, core_ids=list(range(N_CORES)), trace=_trace
    )
    _CACHED["last_results"] = res

    out = np.empty((2, N_SEQ, EMBED), dtype=np.float32)
    for b in range(2):
        acc = res.results[4 * b]["outT"].astype(np.float32).copy()
        for g in range(1, 4):
            acc += res.results[4 * b + g]["outT"]
        out[b] = acc.T
    return out
